# revision 1
# baseline (speedup 1.0000x reference)
"""Trainium2 Bass kernel for nn_AttentionBlockE3 (segment-softmax GNN attention).

Strategy: host sorts edges by destination node and partitions NODES across the
8 cores (1250 nodes each) so each core owns all edges of its nodes — no
collectives needed. Edges are packed per (core, node-chunk-of-128) into a
fixed budget of T_fix 128-edge tiles (padding edges get cutoff=0 / dst=-5 so
they contribute nothing).

Device program per core (all static addressing, shared by all 8 cores):
  pass 1  stream qk tiles, per-head dot products -> logits w [128, 8] per tile
  chunk   global-max-subtract softmax (per-chunk max C is valid for every node
          in the chunk; exp(w - C) keeps the same normalized weights)
  pass 2  stream v tiles, rhs = [w_exp * v  ||  w_exp], one-hot(dst) matmuls
          accumulate [128 nodes, 488] in PSUM = unnormalized output || denom
  epilog  out = psum[:, :480] / (denom + 1e-30), DMA to the node slice
"""
import numpy as np

E, D, N, H = 200000, 480, 10000, 8
P = 128
NCORES = 8
BLOCKS = [(0, 16), (128, 24), (320, 20)]  # (fused col offset, per-head width)
SCALE = 1.0 / np.sqrt(60.0)


def _plan_shard(dst):
    npc = N // NCORES                       # 1250 nodes per core
    CHUNKS = (npc + P - 1) // P             # 10 windows of <=128 nodes
    order = np.argsort(dst, kind="stable")
    dst_s = dst[order]
    lo = np.array([core * npc + c * P
                   for core in range(NCORES) for c in range(CHUNKS)])
    hi = np.array([core * npc + min((c + 1) * P, npc)
                   for core in range(NCORES) for c in range(CHUNKS)])
    starts = np.searchsorted(dst_s, lo, side="left")
    ends = np.searchsorted(dst_s, hi, side="left")
    counts = ends - starts
    T_fix = int(np.max((counts + P - 1) // P))
    budget = T_fix * P
    gi = np.full((NCORES, CHUNKS, budget), -1, np.int64)
    for wi in range(NCORES * CHUNKS):
        core, c = wi // CHUNKS, wi % CHUNKS
        gi[core, c, :counts[wi]] = order[starts[wi]:ends[wi]]
    return gi.reshape(NCORES, -1), T_fix, CHUNKS, npc


def _pack_core(core, gi, T_fix, CHUNKS, npc, key, value, query, cutoff, dst):
    g = gi[core]
    pad = g < 0
    gc = np.clip(g, 0, E - 1)
    qk = np.ascontiguousarray(np.concatenate([query[gc], key[gc]], axis=1))
    v = np.ascontiguousarray(value[gc])
    cut = (cutoff[gc] * SCALE).astype(np.float32)
    cut[pad] = 0.0
    chunk_of = np.repeat(np.arange(CHUNKS), T_fix * P)
    dstrel = (dst[gc] - (core * npc + chunk_of * P)).astype(np.float32)
    dstrel[pad] = -5.0
    T_tot = CHUNKS * T_fix
    cut2 = np.ascontiguousarray(cut.reshape(T_tot, P).T)
    dstrel2 = np.ascontiguousarray(dstrel.reshape(T_tot, P).T)
    return {"qk": qk, "v": v, "cut": cut2, "dstr": dstrel2}


def _build_program(T_fix, CHUNKS):
    import concourse.bacc as bacc
    import concourse.mybir as mybir
    import concourse.tile as tile
    from concourse import bass_isa

    f32 = mybir.dt.float32
    T_tot = CHUNKS * T_fix

    nc = bacc.Bacc("TRN2", target_bir_lowering=False, debug=False,
                   num_devices=NCORES)
    qk_d = nc.dram_tensor("qk", [T_tot * P, 960], f32, kind="ExternalInput").ap()
    v_d = nc.dram_tensor("v", [T_tot * P, 480], f32, kind="ExternalInput").ap()
    cut_d = nc.dram_tensor("cut", [P, T_tot], f32, kind="ExternalInput").ap()
    dstr_d = nc.dram_tensor("dstr", [P, T_tot], f32, kind="ExternalInput").ap()
    out_d = nc.dram_tensor("out", [CHUNKS * P, 480], f32,
                           kind="ExternalOutput").ap()

    with tile.TileContext(nc) as tc:
        with (
            tc.tile_pool(name="const", bufs=1) as const_pool,
            tc.tile_pool(name="qk", bufs=6) as qk_pool,
            tc.tile_pool(name="prod", bufs=4) as prod_pool,
            tc.tile_pool(name="wblk", bufs=6) as wblk_pool,
            tc.tile_pool(name="w", bufs=2) as w_pool,
            tc.tile_pool(name="v", bufs=6) as v_pool,
            tc.tile_pool(name="rhs", bufs=4) as rhs_pool,
            tc.tile_pool(name="oh", bufs=4) as oh_pool,
            tc.tile_pool(name="stat", bufs=4) as stat_pool,
            tc.tile_pool(name="outp", bufs=3) as out_pool,
            tc.tile_pool(name="psum", bufs=2, space="PSUM") as psum_pool,
        ):
            iota_i = const_pool.tile([P, P], mybir.dt.int32)
            nc.gpsimd.iota(iota_i[:], pattern=[[1, P]], base=0,
                           channel_multiplier=0)
            iota_f = const_pool.tile([P, P], f32)
            nc.vector.tensor_copy(iota_f[:], iota_i[:])
            cut_sb = const_pool.tile([P, T_tot], f32)
            nc.sync.dma_start(out=cut_sb[:], in_=cut_d[:, :])
            dstr_sb = const_pool.tile([P, T_tot], f32)
            nc.sync.dma_start(out=dstr_sb[:], in_=dstr_d[:, :])

            for c in range(CHUNKS):
                w_tile = w_pool.tile([P, T_fix * 8], f32)
                for t in range(T_fix):
                    g = c * T_fix + t
                    qkt = qk_pool.tile([P, 960], f32)
                    nc.sync.dma_start(out=qkt[:], in_=qk_d[g * P:(g + 1) * P, :])
                    prod = prod_pool.tile([P, 480], f32)
                    nc.vector.tensor_mul(prod[:], qkt[:, 0:480], qkt[:, 480:960])
                    wsum = wblk_pool.tile([P, 8], f32)
                    tmp = wblk_pool.tile([P, 8], f32)
                    off, hd = BLOCKS[0]
                    nc.vector.reduce_sum(
                        out=wsum[:],
                        in_=prod[:, off:off + 8 * hd].rearrange(
                            "p (h d) -> p h d", h=8),
                        axis=mybir.AxisListType.X)
                    for off, hd in BLOCKS[1:]:
                        nc.vector.reduce_sum(
                            out=tmp[:],
                            in_=prod[:, off:off + 8 * hd].rearrange(
                                "p (h d) -> p h d", h=8),
                            axis=mybir.AxisListType.X)
                        nc.vector.tensor_add(wsum[:], wsum[:], tmp[:])
                    nc.vector.tensor_mul(
                        w_tile[:, t * 8:(t + 1) * 8], wsum[:],
                        cut_sb[:, g:g + 1].to_broadcast([P, 8]))

                wmax = stat_pool.tile([P, 1], f32)
                nc.vector.reduce_max(out=wmax[:], in_=w_tile[:],
                                     axis=mybir.AxisListType.X)
                cmax = stat_pool.tile([P, 1], f32)
                nc.gpsimd.partition_all_reduce(cmax[:], wmax[:], channels=P,
                                               reduce_op=bass_isa.ReduceOp.max)
                negC = stat_pool.tile([P, 1], f32)
                nc.vector.tensor_scalar_mul(negC[:], cmax[:], -1.0)
                nc.scalar.activation(w_tile[:], w_tile[:],
                                     mybir.ActivationFunctionType.Exp,
                                     bias=negC[:], scale=1.0)

                psum_t = psum_pool.tile([P, 488], f32)
                for t in range(T_fix):
                    g = c * T_fix + t
                    vt = v_pool.tile([P, 480], f32)
                    nc.sync.dma_start(out=vt[:], in_=v_d[g * P:(g + 1) * P, :])
                    rhs = rhs_pool.tile([P, 488], f32)
                    wt = w_tile[:, t * 8:(t + 1) * 8]
                    for off, hd in BLOCKS:
                        nc.vector.tensor_mul(
                            rhs[:, off:off + 8 * hd].rearrange(
                                "p (h d) -> p h d", h=8),
                            vt[:, off:off + 8 * hd].rearrange(
                                "p (h d) -> p h d", h=8),
                            wt.unsqueeze(2).to_broadcast([P, 8, hd]))
                    nc.vector.tensor_copy(rhs[:, 480:488], wt)
                    oh = oh_pool.tile([P, P], f32)
                    nc.vector.tensor_tensor(
                        oh[:], dstr_sb[:, g:g + 1].to_broadcast([P, P]),
                        iota_f[:], op=mybir.AluOpType.is_equal)
                    nc.tensor.matmul(out=psum_t[:], lhsT=oh[:], rhs=rhs[:],
                                     start=(t == 0), stop=(t == T_fix - 1))

                srec = stat_pool.tile([P, 8], f32)
                nc.vector.tensor_scalar_add(srec[:], psum_t[:, 480:488], 1e-30)
                nc.vector.reciprocal(srec[:], srec[:])
                outt = out_pool.tile([P, 480], f32)
                for off, hd in BLOCKS:
                    nc.vector.tensor_mul(
                        outt[:, off:off + 8 * hd].rearrange(
                            "p (h d) -> p h d", h=8),
                        psum_t[:, off:off + 8 * hd].rearrange(
                            "p (h d) -> p h d", h=8),
                        srec.unsqueeze(2).to_broadcast([P, 8, hd]))
                nc.sync.dma_start(out=out_d[c * P:(c + 1) * P, :], in_=outt[:])

    nc.compile()
    return nc


def kernel(key, value, query, edge_weight_cutoff, edge_index, num_nodes):
    key = np.ascontiguousarray(np.asarray(key, dtype=np.float32))
    value = np.ascontiguousarray(np.asarray(value, dtype=np.float32))
    query = np.ascontiguousarray(np.asarray(query, dtype=np.float32))
    cutoff = np.asarray(edge_weight_cutoff, dtype=np.float32)
    dst = np.asarray(edge_index)[1].astype(np.int64)

    gi, T_fix, CHUNKS, npc = _plan_shard(dst)
    in_maps = [_pack_core(core, gi, T_fix, CHUNKS, npc,
                          key, value, query, cutoff, dst)
               for core in range(NCORES)]

    nc = _build_program(T_fix, CHUNKS)

    from concourse.bass_utils import run_bass_kernel_spmd
    res = run_bass_kernel_spmd(nc, in_maps, core_ids=list(range(NCORES)))
    out = np.concatenate([r["out"][:npc] for r in res.results])
    return np.ascontiguousarray(out.astype(np.float32))


if __name__ == "__main__":
    rng = np.random.default_rng(0)
    inputs = {
        "key": rng.standard_normal((E, D)).astype(np.float32),
        "value": rng.standard_normal((E, D)).astype(np.float32),
        "query": rng.standard_normal((E, D)).astype(np.float32),
        "edge_weight_cutoff": rng.random(E).astype(np.float32),
        "edge_index": rng.integers(0, N, (2, E)),
        "num_nodes": N,
    }
    out = kernel(**inputs)
    print("out", out.shape, out.dtype, float(np.abs(out).max()))


# revision 4
# speedup vs baseline: 161.2478x; 161.2478x over previous
"""Trainium2 Bass kernel for nn_AttentionBlockE3 (segment-softmax GNN attention).

Strategy: host sorts edges by destination node and partitions NODES across the
8 cores (1250 nodes each) so each core owns all edges of its nodes — no
collectives needed. Edges are packed per (core, node-chunk-of-128) into a
fixed budget of T_fix 128-edge tiles (padding edges get cutoff=0 / dst=-5 so
they contribute nothing).

Device program per core (all static addressing, shared by all 8 cores):
  pass 1  stream qk tiles, per-head dot products -> logits w [128, 8] per tile
  chunk   global-max-subtract softmax (per-chunk max C is valid for every node
          in the chunk; exp(w - C) keeps the same normalized weights)
  pass 2  stream v tiles, rhs = [w_exp * v  ||  w_exp], one-hot(dst) matmuls
          accumulate [128 nodes, 488] in PSUM = unnormalized output || denom
  epilog  out = psum[:, :480] / (denom + 1e-30), DMA to the node slice
"""
import numpy as np

E, D, N, H = 200000, 480, 10000, 8
P = 128
NCORES = 8
BLOCKS = [(0, 16), (128, 24), (320, 20)]  # (fused col offset, per-head width)
SCALE = 1.0 / np.sqrt(60.0)


def _plan_shard(dst):
    npc = N // NCORES                       # 1250 nodes per core
    CHUNKS = (npc + P - 1) // P             # 10 windows of <=128 nodes
    order = np.argsort(dst, kind="stable")
    dst_s = dst[order]
    lo = np.array([core * npc + c * P
                   for core in range(NCORES) for c in range(CHUNKS)])
    hi = np.array([core * npc + min((c + 1) * P, npc)
                   for core in range(NCORES) for c in range(CHUNKS)])
    starts = np.searchsorted(dst_s, lo, side="left")
    ends = np.searchsorted(dst_s, hi, side="left")
    counts = ends - starts
    T_fix = int(np.max((counts + P - 1) // P))
    budget = T_fix * P
    gi = np.full((NCORES, CHUNKS, budget), -1, np.int64)
    for wi in range(NCORES * CHUNKS):
        core, c = wi // CHUNKS, wi % CHUNKS
        gi[core, c, :counts[wi]] = order[starts[wi]:ends[wi]]
    return gi.reshape(NCORES, -1), T_fix, CHUNKS, npc


def _pack_core(core, gi, T_fix, CHUNKS, npc, key, value, query, cutoff, dst):
    g = gi[core]
    pad = g < 0
    gc = np.clip(g, 0, E - 1)
    qk = np.ascontiguousarray(np.concatenate([query[gc], key[gc]], axis=1))
    v = np.ascontiguousarray(value[gc])
    cut = (cutoff[gc] * SCALE).astype(np.float32)
    cut[pad] = 0.0
    chunk_of = np.repeat(np.arange(CHUNKS), T_fix * P)
    dstrel = (dst[gc] - (core * npc + chunk_of * P)).astype(np.float32)
    dstrel[pad] = -5.0
    T_tot = CHUNKS * T_fix
    cut2 = np.ascontiguousarray(cut.reshape(T_tot, P).T)
    dstrel2 = np.ascontiguousarray(dstrel.reshape(T_tot, P).T)
    return {"qk": qk, "v": v, "cut": cut2, "dstr": dstrel2}


def _build_program(T_fix, CHUNKS, reps=1):
    import contextlib

    import concourse.bacc as bacc
    import concourse.mybir as mybir
    import concourse.tile as tile
    from concourse import bass_isa

    f32 = mybir.dt.float32
    T_tot = CHUNKS * T_fix

    nc = bacc.Bacc("TRN2", target_bir_lowering=False, debug=False,
                   num_devices=NCORES)
    qk_d = nc.dram_tensor("qk", [T_tot * P, 960], f32, kind="ExternalInput").ap()
    v_d = nc.dram_tensor("v", [T_tot * P, 480], f32, kind="ExternalInput").ap()
    cut_d = nc.dram_tensor("cut", [P, T_tot], f32, kind="ExternalInput").ap()
    dstr_d = nc.dram_tensor("dstr", [P, T_tot], f32, kind="ExternalInput").ap()
    out_d = nc.dram_tensor("out", [CHUNKS * P, 480], f32,
                           kind="ExternalOutput").ap()

    with tile.TileContext(nc) as tc:
        with (
            tc.tile_pool(name="const", bufs=1) as const_pool,
            tc.tile_pool(name="qk", bufs=6) as qk_pool,
            tc.tile_pool(name="prod", bufs=4) as prod_pool,
            tc.tile_pool(name="wblk", bufs=6) as wblk_pool,
            tc.tile_pool(name="w", bufs=2) as w_pool,
            tc.tile_pool(name="v", bufs=6) as v_pool,
            tc.tile_pool(name="rhs", bufs=4) as rhs_pool,
            tc.tile_pool(name="oh", bufs=4) as oh_pool,
            tc.tile_pool(name="stat", bufs=4) as stat_pool,
            tc.tile_pool(name="outp", bufs=3) as out_pool,
            tc.tile_pool(name="psum", bufs=2, space="PSUM") as psum_pool,
        ):
            iota_i = const_pool.tile([P, P], mybir.dt.int32)
            nc.gpsimd.iota(iota_i[:], pattern=[[1, P]], base=0,
                           channel_multiplier=0)
            iota_f = const_pool.tile([P, P], f32)
            nc.vector.tensor_copy(iota_f[:], iota_i[:])
            cut_sb = const_pool.tile([P, T_tot], f32)
            nc.sync.dma_start(out=cut_sb[:], in_=cut_d[:, :])
            dstr_sb = const_pool.tile([P, T_tot], f32)
            nc.sync.dma_start(out=dstr_sb[:], in_=dstr_d[:, :])

            def chunk_body(c):
                w_tile = w_pool.tile([P, T_fix * 8], f32)
                for t in range(T_fix):
                    g = c * T_fix + t
                    qkt = qk_pool.tile([P, 960], f32)
                    nc.sync.dma_start(out=qkt[:], in_=qk_d[g * P:(g + 1) * P, :])
                    prod = prod_pool.tile([P, 480], f32)
                    nc.vector.tensor_mul(prod[:], qkt[:, 0:480], qkt[:, 480:960])
                    wsum = wblk_pool.tile([P, 8], f32)
                    tmp = wblk_pool.tile([P, 8], f32)
                    off, hd = BLOCKS[0]
                    nc.vector.reduce_sum(
                        out=wsum[:],
                        in_=prod[:, off:off + 8 * hd].rearrange(
                            "p (h d) -> p h d", h=8),
                        axis=mybir.AxisListType.X)
                    for off, hd in BLOCKS[1:]:
                        nc.vector.reduce_sum(
                            out=tmp[:],
                            in_=prod[:, off:off + 8 * hd].rearrange(
                                "p (h d) -> p h d", h=8),
                            axis=mybir.AxisListType.X)
                        nc.vector.tensor_add(wsum[:], wsum[:], tmp[:])
                    nc.vector.tensor_mul(
                        w_tile[:, t * 8:(t + 1) * 8], wsum[:],
                        cut_sb[:, g:g + 1].to_broadcast([P, 8]))

                wmax = stat_pool.tile([P, 1], f32)
                nc.vector.reduce_max(out=wmax[:], in_=w_tile[:],
                                     axis=mybir.AxisListType.X)
                cmax = stat_pool.tile([P, 1], f32)
                nc.gpsimd.partition_all_reduce(cmax[:], wmax[:], channels=P,
                                               reduce_op=bass_isa.ReduceOp.max)
                negC = stat_pool.tile([P, 1], f32)
                nc.vector.tensor_scalar_mul(negC[:], cmax[:], -1.0)
                nc.scalar.activation(w_tile[:], w_tile[:],
                                     mybir.ActivationFunctionType.Exp,
                                     bias=negC[:], scale=1.0)

                psum_t = psum_pool.tile([P, 488], f32)
                for t in range(T_fix):
                    g = c * T_fix + t
                    vt = v_pool.tile([P, 480], f32)
                    nc.sync.dma_start(out=vt[:], in_=v_d[g * P:(g + 1) * P, :])
                    rhs = rhs_pool.tile([P, 488], f32)
                    wt = w_tile[:, t * 8:(t + 1) * 8]
                    for off, hd in BLOCKS:
                        nc.vector.tensor_mul(
                            rhs[:, off:off + 8 * hd].rearrange(
                                "p (h d) -> p h d", h=8),
                            vt[:, off:off + 8 * hd].rearrange(
                                "p (h d) -> p h d", h=8),
                            wt.unsqueeze(2).to_broadcast([P, 8, hd]))
                    nc.vector.tensor_copy(rhs[:, 480:488], wt)
                    oh = oh_pool.tile([P, P], f32)
                    nc.vector.tensor_tensor(
                        oh[:], dstr_sb[:, g:g + 1].to_broadcast([P, P]),
                        iota_f[:], op=mybir.AluOpType.is_equal)
                    nc.tensor.matmul(out=psum_t[:], lhsT=oh[:], rhs=rhs[:],
                                     start=(t == 0), stop=(t == T_fix - 1))

                srec = stat_pool.tile([P, 8], f32)
                nc.vector.tensor_scalar_add(srec[:], psum_t[:, 480:488], 1e-30)
                nc.vector.reciprocal(srec[:], srec[:])
                outt = out_pool.tile([P, 480], f32)
                for off, hd in BLOCKS:
                    nc.vector.tensor_mul(
                        outt[:, off:off + 8 * hd].rearrange(
                            "p (h d) -> p h d", h=8),
                        psum_t[:, off:off + 8 * hd].rearrange(
                            "p (h d) -> p h d", h=8),
                        srec.unsqueeze(2).to_broadcast([P, 8, hd]))
                nc.sync.dma_start(out=out_d[c * P:(c + 1) * P, :], in_=outt[:])

            # reps>1 wraps the body in a hardware loop purely for timing
            # (each rep recomputes and rewrites the identical output)
            loop = tc.For_i(0, reps, 1) if reps > 1 else contextlib.nullcontext()
            with loop:
                for c in range(CHUNKS):
                    chunk_body(c)

    nc.compile()
    return nc


def kernel(key, value, query, edge_weight_cutoff, edge_index, num_nodes):
    key = np.ascontiguousarray(np.asarray(key, dtype=np.float32))
    value = np.ascontiguousarray(np.asarray(value, dtype=np.float32))
    query = np.ascontiguousarray(np.asarray(query, dtype=np.float32))
    cutoff = np.asarray(edge_weight_cutoff, dtype=np.float32)
    dst = np.asarray(edge_index)[1].astype(np.int64)

    gi, T_fix, CHUNKS, npc = _plan_shard(dst)
    in_maps = [_pack_core(core, gi, T_fix, CHUNKS, npc,
                          key, value, query, cutoff, dst)
               for core in range(NCORES)]

    nc = _build_program(T_fix, CHUNKS)

    from concourse.bass_utils import run_bass_kernel_spmd
    res = run_bass_kernel_spmd(nc, in_maps, core_ids=list(range(NCORES)))
    out = np.concatenate([r["out"][:npc] for r in res.results])
    return np.ascontiguousarray(out.astype(np.float32))


if __name__ == "__main__":
    rng = np.random.default_rng(0)
    inputs = {
        "key": rng.standard_normal((E, D)).astype(np.float32),
        "value": rng.standard_normal((E, D)).astype(np.float32),
        "query": rng.standard_normal((E, D)).astype(np.float32),
        "edge_weight_cutoff": rng.random(E).astype(np.float32),
        "edge_index": rng.integers(0, N, (2, E)),
        "num_nodes": N,
    }
    out = kernel(**inputs)
    print("out", out.shape, out.dtype, float(np.abs(out).max()))
